# revision 16
# baseline (speedup 1.0000x reference)
"""Trainium2 Bass kernel for nn_AttentionModule_16398185136487.

Math (the reference reduces to this — its trailing softmax is over a size-1
axis, i.e. ones):
  out = concat([x34, a_x4, x43, b_x3], axis=1)            # (8, 512, 32, 32)
  block(qs, ks, v) = gate(qs, ks) * (w128@wv @ x_v + w128@bv) + b128
  gate(qs, ks)[b, hw] = softmax_hw( (1/8) sum_{kb} max_{khw}
                                    (Q_qs[b,hw] . K_ks[kb,khw]) / 16 )

Sharding: core j owns batch image j (its 1024 query pixels for both the x4
and x3 streams) — the per-image softmax is then fully core-local; no
collectives.  The K tensors (all 16 key images) are computed replicated on
every core from the full x4/x3 (a small 1x1 conv).

Engine plan per core:
  - Q/K convs in bf16, V conv in fp32 (weights fused host-side: w128@wv).
  - scores: 1024 bf16 matmuls (contiguous accumulate pairs over C=256) into
    paired PSUM tiles t0=[A.k0|B.k0], t1=[A.k1|B.k1]; ScalarE stages t0 to
    SBUF while the PE fills t1; a custom DVE op (TTMAX_REDUCE) consumes the
    two 512-wide operands in one pass, producing each image's max.
  - the score loop is key-image-pair OUTER (16 q-tiles inner) with the K conv
    for each image pair emitted just before its consumers, so the PE ramps up
    as soon as the first K image arrives from HBM instead of waiting for the
    whole 8 MB K stream.
  - per-image softmax without max-subtraction (logits are O(1)), gates
    broadcast to 128 partitions via K=1 PE matmuls, and a second custom DVE
    op (GMUL_BIAS) applies out = gate_row * V * (1/S) + b128 in one pass.
    Gates pair (aa, ba) — complete once the 8 x4-key images are consumed —
    runs overlapped with the x3-key score groups; only (ab, bb) is a tail.
"""

import numpy as np
import ml_dtypes

B = 8
C = 256
HW = 1024          # 32*32
BHW = B * HW       # 8192
NCORES = 8

_CACHE = {}


def _ref_ttmax(in0, in1, c0, c1, c2):
    b = np.maximum(in0.astype(np.float32), in1.astype(np.float32))
    return b, np.maximum(c0, b.reshape(b.shape[0], -1).max(axis=-1, keepdims=True))


def _ref_gmul_bias(in0, in1, c0, c1, c2):
    return (in0.astype(np.float32) * in1 * c1 + c0).astype(np.float32)


def _get_custom_ops():
    """Register two custom DVE microcode ops (the ISA-level
    TENSOR_TENSOR_REDUCE crashes this hardware, so we ship our own):

      TTMAX_REDUCE: out = max(in0, in1); accum_out = max(s0, max_k out)
      GMUL_BIAS:    out = in0 * in1 * s1 + s0     (s0, s1 per-partition APs)
    """
    if "ops" in _CACHE:
        return _CACHE["ops"]
    import concourse.dve_ops as dve_ops
    from concourse.dve_ops import DveOp
    from concourse.dve_spec import Spec, Src0, Src1, C0, C1, maxx, lower
    from concourse.dve_uop import DveOpSpec

    def register(name, spec):
        for op in dve_ops.OPS:
            if op.name == name:
                return op
        shas = {}
        for ver in ("v3", "v4"):
            shas[ver] = DveOpSpec(name=name, opcode=1,
                                  uops=lower(spec, ver=ver),
                                  rd1_en=True).sha(ver)
        op = DveOp(name, spec, subdim=False, uops_sha=shas)
        dve_ops.OPS.append(op)
        dve_ops.CUSTOM_DVE_SPECS[op.name] = op.spec
        dve_ops._SUB_OPCODE_FOR_NAME[op.name] = (
            dve_ops._CUSTOM_DVE_ROW_BASE + len(dve_ops.OPS) - 1)
        assert max(dve_ops._SUB_OPCODE_FOR_NAME.values()) < 0x20
        return op

    ttmax = register("TTMAX_REDUCE",
                     Spec(body=maxx(Src0, Src1), accum=maxx, accum_init=C0,
                          reference=_ref_ttmax))
    gmul = register("GMUL_BIAS",
                    Spec(body=Src0 * Src1 * C1 + C0,
                         reference=_ref_gmul_bias))
    _CACHE["ops"] = (ttmax, gmul)
    return _CACHE["ops"]


def _build_nc():
    from contextlib import ExitStack

    import concourse.bass as bass
    import concourse.mybir as mybir
    import concourse.tile as tile
    from concourse import bacc
    from concourse.masks import make_identity

    f32 = mybir.dt.float32
    bf16 = mybir.dt.bfloat16
    fp8 = mybir.dt.float8e4
    AX = mybir.AxisListType.X
    Exp = mybir.ActivationFunctionType.Exp
    Ident = mybir.ActivationFunctionType.Identity

    ttmax, gmul = _get_custom_ops()
    nc = bacc.Bacc("TRN2", target_bir_lowering=False, debug=False,
                   enable_asserts=False, num_devices=NCORES)

    # DRAM I/O (per core)
    x4b_ap = nc.dram_tensor("x4b", (C, BHW), bf16, kind="ExternalInput").ap()
    x3b_ap = nc.dram_tensor("x3b", (C, BHW), bf16, kind="ExternalInput").ap()
    xq_ap = nc.dram_tensor("xq", (C, 2 * HW), bf16, kind="ExternalInput").ap()
    xv_ap = nc.dram_tensor("xv", (C, 2 * HW), f32, kind="ExternalInput").ap()
    wqT_ap = nc.dram_tensor("wqT", (C, C), bf16, kind="ExternalInput").ap()
    wkT_ap = nc.dram_tensor("wkT", (C, C), bf16, kind="ExternalInput").ap()
    wvT_ap = nc.dram_tensor("wvT", (C, 128), f32, kind="ExternalInput").ap()
    bq_ap = nc.dram_tensor("bq", (C, 1), f32, kind="ExternalInput").ap()
    bk_ap = nc.dram_tensor("bk", (C, 1), f32, kind="ExternalInput").ap()
    bvb_ap = nc.dram_tensor("bvb", (128, 1), f32, kind="ExternalInput").ap()
    b128_ap = nc.dram_tensor("b128", (128, 1), f32, kind="ExternalInput").ap()
    out_ap = nc.dram_tensor("out", (512, HW), f32, kind="ExternalOutput").ap()

    SCALE_EFF = (1.0 / 16.0) / 8.0 / 16.0  # /sqrt(C), /8 mean, /16 fp8 pre-scale

    with tile.TileContext(nc) as tc:
        with ExitStack() as ctx:
            const = ctx.enter_context(tc.tile_pool(name="const", bufs=1))
            xs = ctx.enter_context(tc.tile_pool(name="xs", bufs=6))
            ps_pool = ctx.enter_context(
                tc.tile_pool(name="ps", bufs=3, space="PSUM"))
            gps_pool = ctx.enter_context(
                tc.tile_pool(name="gps", bufs=2, space="PSUM"))
            scr = ctx.enter_context(tc.tile_pool(name="scr", bufs=3))
            gp = ctx.enter_context(tc.tile_pool(name="gp", bufs=2))
            fin = ctx.enter_context(tc.tile_pool(name="fin", bufs=2))

            # ---- weights / constants (queue-critical first) ----
            wq_sb, bq_sb, xq_sb = [], [], []
            for ci in range(2):
                w = const.tile([128, C], bf16, tag=f"wq{ci}", name=f"wq{ci}")
                nc.sync.dma_start(w[:], wqT_ap[ci * 128:(ci + 1) * 128, :])
                wq_sb.append(w)
                t = const.tile([128, 2 * HW], bf16, tag=f"xq{ci}", name=f"xq{ci}")
                nc.sync.dma_start(t[:], xq_ap[ci * 128:(ci + 1) * 128, :])
                xq_sb.append(t)
                b = const.tile([128, 1], f32, tag=f"bq{ci}", name=f"bq{ci}")
                nc.gpsimd.dma_start(b[:], bq_ap[ci * 128:(ci + 1) * 128, :])
                bq_sb.append(b)
            wk_sb, bk_sb = [], []
            for ci in range(2):
                w = const.tile([128, C], bf16, tag=f"wk{ci}", name=f"wk{ci}")
                nc.scalar.dma_start(w[:], wkT_ap[ci * 128:(ci + 1) * 128, :])
                wk_sb.append(w)
                b = const.tile([128, 1], f32, tag=f"bk{ci}", name=f"bk{ci}")
                nc.gpsimd.dma_start(b[:], bk_ap[ci * 128:(ci + 1) * 128, :])
                bk_sb.append(b)
            bvb_sb = const.tile([128, 1], f32, tag="bvb", name="bvb")
            nc.gpsimd.dma_start(bvb_sb[:], bvb_ap[:, :])
            b128_sb = const.tile([128, 1], f32, tag="b128", name="b128")
            nc.gpsimd.dma_start(b128_sb[:], b128_ap[:, :])

            ones_row = const.tile([1, 128], f32, tag="ones_row", name="ones_row")
            nc.vector.memset(ones_row[:], 1.0)
            ones_col = const.tile([128, 1], f32, tag="ones_col", name="ones_col")
            nc.vector.memset(ones_col[:], 1.0)
            ident = const.tile([128, 128], f32, tag="ident", name="ident")
            make_identity(nc, ident[:])

            # residents produced on device (fp8 for the DoubleRow GEMM;
            # 2D tiles viewed as [128, cin_chunk, pix])
            q8 = const.tile([128, 2 * 2 * HW], fp8, tag="q8", name="q8")
            ka8 = const.tile([128, 2 * BHW], fp8, tag="ka8", name="ka8")
            kb8 = const.tile([128, 2 * BHW], fp8, tag="kb8", name="kb8")
            q3 = q8.rearrange("p (s n) -> p s n", s=2)
            ka3 = ka8.rearrange("p (s n) -> p s n", s=2)
            kb3 = kb8.rearrange("p (s n) -> p s n", s=2)
            va_sb = const.tile([128, HW], f32, tag="va", name="va")
            vb_sb = const.tile([128, HW], f32, tag="vb", name="vb")

            # per-q-tile max accumulators (col = key image 0..15)
            m16s = [const.tile([128, 16], f32, tag=f"m16_{qs}", name=f"m16_{qs}")
                    for qs in range(16)]
            # M pair tiles: cols 0:8 = first gate (qs 0..7), 8:16 = second
            Mka = const.tile([128, 16], f32, tag="Mka", name="Mka")  # (aa, ba)
            Mkb = const.tile([128, 16], f32, tag="Mkb", name="Mkb")  # (ab, bb)

            # ---- Q conv (bf16) ----
            for co in range(2):
                for n2 in range(2):
                    qps = ps_pool.tile([128, 1024], f32, tag="ps", name="qps")
                    for half in range(2):
                        n = n2 * 2 + half
                        for ci in range(2):
                            nc.tensor.matmul(
                                qps[:, half * 512:(half + 1) * 512],
                                wq_sb[ci][:, co * 128:(co + 1) * 128],
                                xq_sb[ci][:, n * 512:(n + 1) * 512],
                                start=(ci == 0), stop=(ci == 1))
                    nc.scalar.activation(
                        q8[:, co * 2048 + n2 * 1024:co * 2048 + (n2 + 1) * 1024],
                        qps[:, :], Ident, bias=bq_sb[co][:], scale=4.0)

            # ---- K conv for one key image (1024 cols of one stream) ----
            def emit_kconv(img):
                src_eng = nc.sync if img < 8 else nc.scalar
                src_ap = x4b_ap if img < 8 else x3b_ap
                k8 = ka8 if img < 8 else kb8
                n2 = img % 8
                xt = []
                for ci in range(2):
                    t = xs.tile([128, 1024], bf16, tag="xt", name="xt")
                    src_eng.dma_start(
                        t[:], src_ap[ci * 128:(ci + 1) * 128,
                                     n2 * 1024:(n2 + 1) * 1024])
                    xt.append(t)
                for co in range(2):
                    kps = ps_pool.tile([128, 1024], f32, tag="ps", name="kps")
                    for half in range(2):
                        for ci in range(2):
                            nc.tensor.matmul(
                                kps[:, half * 512:(half + 1) * 512],
                                wk_sb[ci][:, co * 128:(co + 1) * 128],
                                xt[ci][:, half * 512:(half + 1) * 512],
                                start=(ci == 0), stop=(ci == 1))
                    nc.scalar.activation(
                        k8[:, co * BHW + n2 * 1024:co * BHW + (n2 + 1) * 1024],
                        kps[:, :], Ident, bias=bk_sb[co][:], scale=4.0)

            # ---- V conv (fp32, fused weights) ----
            def emit_vconv():
                wv_sb, xv_sb = [], []
                for ci in range(2):
                    w = const.tile([128, 128], f32, tag=f"wv{ci}", name=f"wv{ci}")
                    nc.gpsimd.dma_start(w[:], wvT_ap[ci * 128:(ci + 1) * 128, :])
                    wv_sb.append(w)
                    t = const.tile([128, 2 * HW], f32, tag=f"xv{ci}",
                                   name=f"xv{ci}")
                    nc.gpsimd.dma_start(t[:], xv_ap[ci * 128:(ci + 1) * 128, :])
                    xv_sb.append(t)
                for st, v_sb in ((0, va_sb), (1, vb_sb)):
                    vps = ps_pool.tile([128, 1024], f32, tag="ps", name="vps")
                    for half in range(2):
                        for ci in range(2):
                            nc.tensor.matmul(
                                vps[:, half * 512:(half + 1) * 512],
                                wv_sb[ci][:, :],
                                xv_sb[ci][:, st * HW + half * 512:
                                           st * HW + (half + 1) * 512],
                                start=(ci == 0), stop=(ci == 1))
                    nc.scalar.activation(v_sb[:, :], vps[:, :], Ident,
                                         bias=bvb_sb[:])

            # ---- scores for one (q-tile, key-image-pair) ----
            def emit_scores(qs, grp):
                ia, ib = grp * 2, grp * 2 + 1
                qcol = qs * 128
                t0 = ps_pool.tile([128, 1024], f32, tag="ps", name="t0")
                t1 = ps_pool.tile([128, 1024], f32, tag="ps", name="t1")

                def score_mms(tdst, half, second):
                    img = ia if half == 0 else ib
                    k3 = ka3 if img < 8 else kb3
                    kcol = (img % 8) * HW + (512 if second else 0)
                    nc.tensor.matmul(
                        tdst[:, half * 512:(half + 1) * 512],
                        q3[:, :, qcol:qcol + 128],
                        k3[:, :, kcol:kcol + 512],
                        start=True, stop=True,
                        perf_mode=mybir.MatmulPerfMode.DoubleRow)

                score_mms(t0, 0, False)
                score_mms(t0, 1, False)
                # Stage the earlier-finished tile so its banks free while the
                # PE is still filling t1 (avoids PSUM-slot stalls).
                cp = scr.tile([128, 1024], f32, tag="cp", name="cp", bufs=3)
                nc.scalar.copy(cp[:], t0[:, :])
                score_mms(t1, 0, True)
                score_mms(t1, 1, True)
                m16 = m16s[qs]
                for half, img in ((0, ia), (1, ib)):
                    sc = scr.tile([128, 512], f32, tag="sc", name="sc", bufs=3)
                    nc.vector._custom_dve(
                        ttmax, out=sc[:],
                        in0=t1[:, half * 512:(half + 1) * 512],
                        in1=cp[:, half * 512:(half + 1) * 512],
                        s0=-3.0e38, accum_out=m16[:, img:img + 1])

            # ---- batched softmax + apply for a pair of gates ----
            def emit_gate_pair(Mpair, specs, tagp):
                E2 = gp.tile([128, 16], f32, tag=f"E2{tagp}", name="E2")
                nc.scalar.activation(E2[:], Mpair[:], Exp, bias=0.0,
                                     scale=SCALE_EFF)
                sr = gp.tile([128, 2], f32, tag=f"sr{tagp}", name="sr")
                nc.vector.reduce_sum(
                    sr[:], E2.rearrange("p (g k) -> p g k", g=2), axis=AX)
                sum_ps = gps_pool.tile([128, 512], f32, tag="gps", name="sum_ps")
                nc.tensor.matmul(sum_ps[0:2, 0:1], sr[:], ones_col[:],
                                 start=True, stop=True)
                rec2 = gp.tile([2, 1], f32, tag=f"rec{tagp}", name="rec2")
                nc.vector.reciprocal(rec2[:], sum_ps[0:2, 0:1])
                tp = gps_pool.tile([128, 512], f32, tag="gps", name="tp")
                nc.tensor.transpose(tp[0:1, 0:2], rec2[:], ident[0:2, 0:2])
                recT = gp.tile([1, 2], f32, tag=f"recT{tagp}", name="recT")
                nc.scalar.copy(recT[:], tp[0:1, 0:2])
                bc = gps_pool.tile([128, 512], f32, tag="gps", name="bc")
                nc.tensor.matmul(bc[:, 0:2], ones_row[:], recT[:],
                                 start=True, stop=True)
                rsb2 = gp.tile([128, 2], f32, tag=f"rsb{tagp}", name="rsb2")
                nc.scalar.copy(rsb2[:], bc[:, 0:2])
                # transpose E (128,16) -> (16,128), flatten to a (1,2048) row
                tpe = gps_pool.tile([128, 512], f32, tag="gps", name="tpe")
                nc.tensor.transpose(tpe[0:16, 0:128], E2[:], ident[:])
                et = gp.tile([16, 128], f32, tag=f"et{tagp}", name="et")
                nc.scalar.copy(et[:], tpe[0:16, 0:128])
                grow = gp.tile([1, 2048], f32, tag=f"grow{tagp}", name="grow")
                nc.sync.dma_start(grow.rearrange("a (t p) -> a t p", t=16),
                                  et[:])
                for gidx, (v_sb, blk) in enumerate(specs):
                    out_t = fin.tile([128, HW], f32, tag="out_t", name="out_t")
                    for half in range(2):
                        gb = gps_pool.tile([128, 512], f32, tag="gps",
                                           name="gb")
                        nc.tensor.matmul(
                            gb[:, :], ones_row[:],
                            grow[0:1, gidx * 1024 + half * 512:
                                 gidx * 1024 + (half + 1) * 512],
                            start=True, stop=True)
                        nc.vector._custom_dve(
                            gmul, out=out_t[:, half * 512:(half + 1) * 512],
                            in0=gb[:, :],
                            in1=v_sb[:, half * 512:(half + 1) * 512],
                            s0=b128_sb[:], s1=rsb2[:, gidx:gidx + 1])
                    eng = nc.sync if gidx == 0 else nc.scalar
                    eng.dma_start(out_ap[blk * 128:(blk + 1) * 128, :],
                                  out_t[:])

            # ---- main schedule ----
            for grp in range(4):               # x4-stream key images 0..7
                emit_kconv(2 * grp)
                emit_kconv(2 * grp + 1)
                if grp == 0:
                    emit_vconv()
                for qs in range(16):
                    emit_scores(qs, grp)
            for qs in range(16):
                nc.vector.reduce_sum(Mka[:, qs:qs + 1], m16s[qs][:, 0:8],
                                     axis=AX)
            # (aa -> block 1, ba -> block 0); overlaps the x3-key score groups
            emit_gate_pair(Mka, [(va_sb, 1), (va_sb, 0)], "1")
            for grp in range(4, 8):            # x3-stream key images 8..15
                emit_kconv(2 * grp)
                emit_kconv(2 * grp + 1)
                for qs in range(16):
                    emit_scores(qs, grp)
            for qs in range(16):
                nc.vector.reduce_sum(Mkb[:, qs:qs + 1], m16s[qs][:, 8:16],
                                     axis=AX)
            # (ab -> block 2, bb -> block 3)
            emit_gate_pair(Mkb, [(vb_sb, 2), (vb_sb, 3)], "2")

    nc.compile()
    return nc


def get_nc():
    if "nc" not in _CACHE:
        _CACHE["nc"] = _build_nc()
    return _CACHE["nc"]


def prepare_in_maps(x4, x3, wq, bq, wk, bk, wv, bv, w128, b128):
    bf16 = ml_dtypes.bfloat16
    x4 = np.asarray(x4, np.float32)
    x3 = np.asarray(x3, np.float32)
    X4 = np.ascontiguousarray(x4.transpose(1, 0, 2, 3).reshape(C, BHW))
    X3 = np.ascontiguousarray(x3.transpose(1, 0, 2, 3).reshape(C, BHW))
    X4b = X4.astype(bf16)
    X3b = X3.astype(bf16)
    wq = np.asarray(wq, np.float32)
    wk = np.asarray(wk, np.float32)
    wv = np.asarray(wv, np.float32)
    w128 = np.asarray(w128, np.float32)
    wqT = np.ascontiguousarray(wq.T).astype(bf16)
    wkT = np.ascontiguousarray(wk.T).astype(bf16)
    wvT = np.ascontiguousarray((w128 @ wv).T)          # (256, 128) f32
    bq2 = 4.0 * np.asarray(bq, np.float32).reshape(C, 1)
    bk2 = 4.0 * np.asarray(bk, np.float32).reshape(C, 1)
    bvb = (w128 @ np.asarray(bv, np.float32)).reshape(128, 1).astype(np.float32)
    b128r = np.asarray(b128, np.float32).reshape(128, 1)

    in_maps = []
    for j in range(NCORES):
        sl = slice(j * HW, (j + 1) * HW)
        xq = np.concatenate([X4b[:, sl], X3b[:, sl]], axis=1)
        xv = np.concatenate([X4[:, sl], X3[:, sl]], axis=1)
        in_maps.append({
            "x4b": X4b, "x3b": X3b,
            "xq": np.ascontiguousarray(xq),
            "xv": np.ascontiguousarray(xv),
            "wqT": wqT, "wkT": wkT, "wvT": wvT,
            "bq": bq2, "bk": bk2, "bvb": bvb, "b128": b128r,
        })
    return in_maps


def kernel(**inputs):
    from concourse.bass_utils import run_bass_kernel_spmd

    nc = get_nc()
    in_maps = prepare_in_maps(**inputs)
    res = run_bass_kernel_spmd(nc, in_maps, core_ids=list(range(NCORES)))
    out = np.stack([res.results[c]["out"].reshape(512, 32, 32)
                    for c in range(NCORES)])
    return np.ascontiguousarray(out.astype(np.float32))


# revision 24
# speedup vs baseline: 1.1374x; 1.1374x over previous
"""Trainium2 Bass kernel for nn_AttentionModule_16398185136487.

Math (the reference reduces to this — its trailing softmax is over a size-1
axis, i.e. ones):
  out = concat([x34, a_x4, x43, b_x3], axis=1)            # (8, 512, 32, 32)
  block(qs, ks, v) = gate(qs, ks) * (w128@wv @ x_v + w128@bv) + b128
  gate(qs, ks)[b, hw] = softmax_hw( (1/8) sum_{kb} max_{khw}
                                    (Q_qs[b,hw] . K_ks[kb,khw]) / 16 )

Sharding: core j owns batch image j (its 1024 query pixels for both the x4
and x3 streams) — the per-image softmax is then fully core-local; no
collectives.  The K tensors (all 16 key images) are computed replicated on
every core from the full x4/x3 (a small 1x1 conv).

Engine plan per core:
  - Q/K convs in bf16, V conv in fp32 (weights fused host-side: w128@wv).
  - scores: 1024 bf16 matmuls (contiguous accumulate pairs over C=256) into
    paired PSUM tiles t0=[A.k0|B.k0], t1=[A.k1|B.k1]; ScalarE stages t0 to
    SBUF while the PE fills t1; a custom DVE op (TTMAX_REDUCE) consumes the
    two 512-wide operands in one pass, producing each image's max.
  - the score loop is key-image-pair OUTER (16 q-tiles inner) with the K conv
    for each image pair emitted just before its consumers, so the PE ramps up
    as soon as the first K image arrives from HBM instead of waiting for the
    whole 8 MB K stream.
  - per-image softmax without max-subtraction (logits are O(1)), gates
    broadcast to 128 partitions via K=1 PE matmuls, and a second custom DVE
    op (GMUL_BIAS) applies out = gate_row * V * (1/S) + b128 in one pass.
    Gates pair (aa, ba) — complete once the 8 x4-key images are consumed —
    runs overlapped with the x3-key score groups; only (ab, bb) is a tail.
"""

import numpy as np
import ml_dtypes

B = 8
C = 256
HW = 1024          # 32*32
BHW = B * HW       # 8192
NCORES = 8

_CACHE = {}


def _ref_ttmax(in0, in1, c0, c1, c2):
    b = np.maximum(in0.astype(np.float32), in1.astype(np.float32))
    return b, np.maximum(c0, b.reshape(b.shape[0], -1).max(axis=-1, keepdims=True))


def _ref_gmul_bias(in0, in1, c0, c1, c2):
    return (in0.astype(np.float32) * in1 * c1 + c0).astype(np.float32)


def _get_custom_ops():
    """Register two custom DVE microcode ops (the ISA-level
    TENSOR_TENSOR_REDUCE crashes this hardware, so we ship our own):

      TTMAX_REDUCE: out = max(in0, in1); accum_out = max(s0, max_k out)
      GMUL_BIAS:    out = in0 * in1 * s1 + s0     (s0, s1 per-partition APs)
    """
    if "ops" in _CACHE:
        return _CACHE["ops"]
    import concourse.dve_ops as dve_ops
    from concourse.dve_ops import DveOp
    from concourse.dve_spec import Spec, Src0, Src1, C0, C1, maxx, lower
    from concourse.dve_uop import DveOpSpec

    def register(name, spec):
        for op in dve_ops.OPS:
            if op.name == name:
                return op
        shas = {}
        for ver in ("v3", "v4"):
            shas[ver] = DveOpSpec(name=name, opcode=1,
                                  uops=lower(spec, ver=ver),
                                  rd1_en=True).sha(ver)
        op = DveOp(name, spec, subdim=False, uops_sha=shas)
        dve_ops.OPS.append(op)
        dve_ops.CUSTOM_DVE_SPECS[op.name] = op.spec
        dve_ops._SUB_OPCODE_FOR_NAME[op.name] = (
            dve_ops._CUSTOM_DVE_ROW_BASE + len(dve_ops.OPS) - 1)
        assert max(dve_ops._SUB_OPCODE_FOR_NAME.values()) < 0x20
        return op

    ttmax = register("TTMAX_REDUCE",
                     Spec(body=maxx(Src0, Src1), accum=maxx, accum_init=C0,
                          reference=_ref_ttmax))
    gmul = register("GMUL_BIAS",
                    Spec(body=Src0 * Src1 * C1 + C0,
                         reference=_ref_gmul_bias))
    _CACHE["ops"] = (ttmax, gmul)
    return _CACHE["ops"]


def _build_nc():
    from contextlib import ExitStack

    import concourse.bass as bass
    import concourse.mybir as mybir
    import concourse.tile as tile
    from concourse import bacc
    from concourse.masks import make_identity

    f32 = mybir.dt.float32
    bf16 = mybir.dt.bfloat16
    fp8 = mybir.dt.float8e4
    AX = mybir.AxisListType.X
    Exp = mybir.ActivationFunctionType.Exp
    Ident = mybir.ActivationFunctionType.Identity

    ttmax, gmul = _get_custom_ops()
    nc = bacc.Bacc("TRN2", target_bir_lowering=False, debug=False,
                   enable_asserts=False, num_devices=NCORES)

    # DRAM I/O (per core)
    x4b_ap = nc.dram_tensor("x4b", (C, BHW), bf16, kind="ExternalInput").ap()
    x3b_ap = nc.dram_tensor("x3b", (C, BHW), bf16, kind="ExternalInput").ap()
    xq_ap = nc.dram_tensor("xq", (C, 2 * HW), bf16, kind="ExternalInput").ap()
    xv_ap = nc.dram_tensor("xv", (C, 2 * HW), f32, kind="ExternalInput").ap()
    wqT_ap = nc.dram_tensor("wqT", (C, C), bf16, kind="ExternalInput").ap()
    wkT_ap = nc.dram_tensor("wkT", (C, C), bf16, kind="ExternalInput").ap()
    wvT_ap = nc.dram_tensor("wvT", (C, 128), f32, kind="ExternalInput").ap()
    bq_ap = nc.dram_tensor("bq", (C, 1), f32, kind="ExternalInput").ap()
    bk_ap = nc.dram_tensor("bk", (C, 1), f32, kind="ExternalInput").ap()
    bvb_ap = nc.dram_tensor("bvb", (128, 1), f32, kind="ExternalInput").ap()
    b128_ap = nc.dram_tensor("b128", (128, 1), f32, kind="ExternalInput").ap()
    out_ap = nc.dram_tensor("out", (512, HW), f32, kind="ExternalOutput").ap()

    SCALE_EFF = (1.0 / 16.0) / 8.0 / 16.0  # /sqrt(C), /8 mean, /16 fp8 pre-scale

    with tile.TileContext(nc) as tc:
        with ExitStack() as ctx:
            const = ctx.enter_context(tc.tile_pool(name="const", bufs=1))
            xs = ctx.enter_context(tc.tile_pool(name="xs", bufs=6))
            ps_pool = ctx.enter_context(
                tc.tile_pool(name="ps", bufs=3, space="PSUM"))
            gps_pool = ctx.enter_context(
                tc.tile_pool(name="gps", bufs=2, space="PSUM"))
            scr = ctx.enter_context(tc.tile_pool(name="scr", bufs=3))
            gp = ctx.enter_context(tc.tile_pool(name="gp", bufs=2))
            fin = ctx.enter_context(tc.tile_pool(name="fin", bufs=2))

            # ---- weights / constants (queue-critical first) ----
            wq_sb, bq_sb, xq_sb = [], [], []
            for ci in range(2):
                w = const.tile([128, C], bf16, tag=f"wq{ci}", name=f"wq{ci}")
                nc.sync.dma_start(w[:], wqT_ap[ci * 128:(ci + 1) * 128, :])
                wq_sb.append(w)
                t = const.tile([128, 2 * HW], bf16, tag=f"xq{ci}", name=f"xq{ci}")
                nc.sync.dma_start(t[:], xq_ap[ci * 128:(ci + 1) * 128, :])
                xq_sb.append(t)
                b = const.tile([128, 1], f32, tag=f"bq{ci}", name=f"bq{ci}")
                nc.gpsimd.dma_start(b[:], bq_ap[ci * 128:(ci + 1) * 128, :])
                bq_sb.append(b)
            wk_sb, bk_sb = [], []
            for ci in range(2):
                w = const.tile([128, C], bf16, tag=f"wk{ci}", name=f"wk{ci}")
                nc.scalar.dma_start(w[:], wkT_ap[ci * 128:(ci + 1) * 128, :])
                wk_sb.append(w)
                b = const.tile([128, 1], f32, tag=f"bk{ci}", name=f"bk{ci}")
                nc.gpsimd.dma_start(b[:], bk_ap[ci * 128:(ci + 1) * 128, :])
                bk_sb.append(b)
            bvb_sb = const.tile([128, 1], f32, tag="bvb", name="bvb")
            nc.gpsimd.dma_start(bvb_sb[:], bvb_ap[:, :])
            b128_sb = const.tile([128, 1], f32, tag="b128", name="b128")
            nc.gpsimd.dma_start(b128_sb[:], b128_ap[:, :])

            ones_row = const.tile([1, 128], f32, tag="ones_row", name="ones_row")
            nc.vector.memset(ones_row[:], 1.0)
            ones_col = const.tile([128, 1], f32, tag="ones_col", name="ones_col")
            nc.vector.memset(ones_col[:], 1.0)
            ident = const.tile([128, 128], f32, tag="ident", name="ident")
            make_identity(nc, ident[:])
            neg96 = const.tile([128, 1], f32, tag="neg96", name="neg96")
            nc.vector.memset(neg96[:], -70.0)

            # residents produced on device (fp8 for the DoubleRow GEMM;
            # 2D tiles viewed as [128, cin_chunk, pix])
            q8 = const.tile([128, 2 * 2 * HW], fp8, tag="q8", name="q8")
            ka8 = const.tile([128, 2 * BHW], fp8, tag="ka8", name="ka8")
            kb8 = const.tile([128, 2 * BHW], fp8, tag="kb8", name="kb8")
            q3 = q8.rearrange("p (s n) -> p s n", s=2)
            ka3 = ka8.rearrange("p (s n) -> p s n", s=2)
            kb3 = kb8.rearrange("p (s n) -> p s n", s=2)
            va_sb = const.tile([128, HW], f32, tag="va", name="va")
            vb_sb = const.tile([128, HW], f32, tag="vb", name="vb")

            # per-q-tile max accumulators (col = key image 0..15);
            # odd columns exact (DVE), even columns via ScalarE LSE whose
            # exp-sums accumulate in lse8s (col = image pair)
            m16s = [const.tile([128, 16], f32, tag=f"m16_{qs}", name=f"m16_{qs}")
                    for qs in range(16)]
            lse8s = [const.tile([128, 8], f32, tag=f"lse8_{qs}",
                                name=f"lse8_{qs}")
                     for qs in range(16)]
            # M pair tiles: cols 0:8 = first gate (qs 0..7), 8:16 = second
            Mka = const.tile([128, 16], f32, tag="Mka", name="Mka")  # (aa, ba)
            Mkb = const.tile([128, 16], f32, tag="Mkb", name="Mkb")  # (ab, bb)

            # ---- Q conv (bf16) ----
            for co in range(2):
                for n2 in range(2):
                    qps = ps_pool.tile([128, 1024], f32, tag="ps", name="qps")
                    for half in range(2):
                        n = n2 * 2 + half
                        for ci in range(2):
                            nc.tensor.matmul(
                                qps[:, half * 512:(half + 1) * 512],
                                wq_sb[ci][:, co * 128:(co + 1) * 128],
                                xq_sb[ci][:, n * 512:(n + 1) * 512],
                                start=(ci == 0), stop=(ci == 1))
                    nc.scalar.activation(
                        q8[:, co * 2048 + n2 * 1024:co * 2048 + (n2 + 1) * 1024],
                        qps[:, :], Ident, bias=bq_sb[co][:], scale=4.0)

            # ---- K conv for one key image (1024 cols of one stream) ----
            def emit_kconv(img):
                src_eng = nc.sync if img < 8 else nc.scalar
                src_ap = x4b_ap if img < 8 else x3b_ap
                k8 = ka8 if img < 8 else kb8
                n2 = img % 8
                xt = []
                for ci in range(2):
                    t = xs.tile([128, 1024], bf16, tag="xt", name="xt")
                    src_eng.dma_start(
                        t[:], src_ap[ci * 128:(ci + 1) * 128,
                                     n2 * 1024:(n2 + 1) * 1024])
                    xt.append(t)
                for co in range(2):
                    kps = ps_pool.tile([128, 1024], f32, tag="ps", name="kps")
                    for half in range(2):
                        for ci in range(2):
                            nc.tensor.matmul(
                                kps[:, half * 512:(half + 1) * 512],
                                wk_sb[ci][:, co * 128:(co + 1) * 128],
                                xt[ci][:, half * 512:(half + 1) * 512],
                                start=(ci == 0), stop=(ci == 1))
                    kdst = k8[:, co * BHW + n2 * 1024:
                              co * BHW + (n2 + 1) * 1024]
                    if co == 0:
                        nc.scalar.activation(kdst, kps[:, :], Ident,
                                             bias=bk_sb[co][:], scale=4.0)
                    else:
                        nc.vector.tensor_scalar(
                            kdst, kps[:, :], 4.0, bk_sb[co][:],
                            op0=mybir.AluOpType.mult,
                            op1=mybir.AluOpType.add)

            # ---- V conv (fp32, fused weights) ----
            def emit_vconv():
                wv_sb, xv_sb = [], []
                for ci in range(2):
                    w = const.tile([128, 128], f32, tag=f"wv{ci}", name=f"wv{ci}")
                    nc.gpsimd.dma_start(w[:], wvT_ap[ci * 128:(ci + 1) * 128, :])
                    wv_sb.append(w)
                    t = const.tile([128, 2 * HW], f32, tag=f"xv{ci}",
                                   name=f"xv{ci}")
                    nc.gpsimd.dma_start(t[:], xv_ap[ci * 128:(ci + 1) * 128, :])
                    xv_sb.append(t)
                for st, v_sb in ((0, va_sb), (1, vb_sb)):
                    vps = ps_pool.tile([128, 1024], f32, tag="ps", name="vps")
                    for half in range(2):
                        for ci in range(2):
                            nc.tensor.matmul(
                                vps[:, half * 512:(half + 1) * 512],
                                wv_sb[ci][:, :],
                                xv_sb[ci][:, st * HW + half * 512:
                                           st * HW + (half + 1) * 512],
                                start=(ci == 0), stop=(ci == 1))
                    nc.scalar.activation(v_sb[:, :], vps[:, :], Ident,
                                         bias=bvb_sb[:])

            # ---- scores for one (q-tile, key-image-pair) ----
            # t0 holds ALL 1024 scores of image ia, t1 of image ib.  The two
            # tiles then go to two INDEPENDENT consumers (no cross-dep, no
            # staging copy):
            #   t0 -> ScalarE log-sum-exp max (exp accum straight from PSUM;
            #         the log happens once per q-tile later; beta_eff = 16 in
            #         true-score units, so the LSE-max error is ~0.03/16 and
            #         constant shifts cancel in the softmax)
            #   t1 -> VectorE exact reduce_max from PSUM
            def emit_scores(qs, grp):
                ia, ib = grp * 2, grp * 2 + 1
                qcol = qs * 128
                t0 = ps_pool.tile([128, 1024], f32, tag="ps", name="t0")
                t1 = ps_pool.tile([128, 1024], f32, tag="ps", name="t1")

                def score_mms(tdst, img, half):
                    k3 = ka3 if img < 8 else kb3
                    kcol = (img % 8) * HW + half * 512
                    nc.tensor.matmul(
                        tdst[:, half * 512:(half + 1) * 512],
                        q3[:, :, qcol:qcol + 128],
                        k3[:, :, kcol:kcol + 512],
                        start=True, stop=True,
                        perf_mode=mybir.MatmulPerfMode.DoubleRow)

                score_mms(t0, ia, 0)
                score_mms(t0, ia, 1)
                lse8 = lse8s[qs]
                ex = scr.tile([128, 1024], f32, tag="ex", name="ex", bufs=3)
                nc.scalar.activation(ex[:], t0[:, :], Exp, bias=neg96[:],
                                     scale=0.5, accum_out=lse8[:, grp:grp + 1])
                score_mms(t1, ib, 0)
                score_mms(t1, ib, 1)
                m16 = m16s[qs]
                nc.vector.reduce_max(m16[:, ib:ib + 1], t1[:, :], axis=AX)

            # ---- batched softmax + apply for a pair of gates ----
            def emit_gate_pair(Mpair, specs, tagp):
                E2 = gp.tile([128, 16], f32, tag=f"E2{tagp}", name="E2")
                nc.scalar.activation(E2[:], Mpair[:], Exp, bias=0.0,
                                     scale=SCALE_EFF)
                sr = gp.tile([128, 2], f32, tag=f"sr{tagp}", name="sr")
                nc.vector.reduce_sum(
                    sr[:], E2.rearrange("p (g k) -> p g k", g=2), axis=AX)
                sum_ps = gps_pool.tile([128, 512], f32, tag="gps", name="sum_ps")
                nc.tensor.matmul(sum_ps[0:2, 0:1], sr[:], ones_col[:],
                                 start=True, stop=True)
                rec2 = gp.tile([2, 1], f32, tag=f"rec{tagp}", name="rec2")
                nc.vector.reciprocal(rec2[:], sum_ps[0:2, 0:1])
                tp = gps_pool.tile([128, 512], f32, tag="gps", name="tp")
                nc.tensor.transpose(tp[0:1, 0:2], rec2[:], ident[0:2, 0:2])
                recT = gp.tile([1, 2], f32, tag=f"recT{tagp}", name="recT")
                nc.scalar.copy(recT[:], tp[0:1, 0:2])
                bc = gps_pool.tile([128, 512], f32, tag="gps", name="bc")
                nc.tensor.matmul(bc[:, 0:2], ones_row[:], recT[:],
                                 start=True, stop=True)
                rsb2 = gp.tile([128, 2], f32, tag=f"rsb{tagp}", name="rsb2")
                nc.scalar.copy(rsb2[:], bc[:, 0:2])
                # transpose E (128,16) -> (16,128), flatten to a (1,2048) row
                tpe = gps_pool.tile([128, 512], f32, tag="gps", name="tpe")
                nc.tensor.transpose(tpe[0:16, 0:128], E2[:], ident[:])
                et = gp.tile([16, 128], f32, tag=f"et{tagp}", name="et")
                nc.scalar.copy(et[:], tpe[0:16, 0:128])
                grow = gp.tile([1, 2048], f32, tag=f"grow{tagp}", name="grow")
                nc.sync.dma_start(grow.rearrange("a (t p) -> a t p", t=16),
                                  et[:])
                for gidx, (v_sb, blk) in enumerate(specs):
                    out_t = fin.tile([128, HW], f32, tag="out_t", name="out_t")
                    for half in range(2):
                        gb = gps_pool.tile([128, 512], f32, tag="gps",
                                           name="gb")
                        nc.tensor.matmul(
                            gb[:, :], ones_row[:],
                            grow[0:1, gidx * 1024 + half * 512:
                                 gidx * 1024 + (half + 1) * 512],
                            start=True, stop=True)
                        nc.vector._custom_dve(
                            gmul, out=out_t[:, half * 512:(half + 1) * 512],
                            in0=gb[:, :],
                            in1=v_sb[:, half * 512:(half + 1) * 512],
                            s0=b128_sb[:], s1=rsb2[:, gidx:gidx + 1])
                    eng = nc.sync if gidx == 0 else nc.scalar
                    eng.dma_start(out_ap[blk * 128:(blk + 1) * 128, :],
                                  out_t[:])

            # ---- main schedule ----
            for grp in range(4):               # x4-stream key images 0..7
                emit_kconv(2 * grp)
                emit_kconv(2 * grp + 1)
                if grp == 0:
                    emit_vconv()
                for qs in range(16):
                    emit_scores(qs, grp)
            Ln = mybir.ActivationFunctionType.Ln
            MULT, ADD = mybir.AluOpType.mult, mybir.AluOpType.add

            u32 = mybir.dt.uint32
            SHR = mybir.AluOpType.logical_shift_right
            BAND = mybir.AluOpType.bitwise_and
            BOR = mybir.AluOpType.bitwise_or

            def emit_msum(Mdst, qs, lo):
                # exact maxes live in odd m16 columns (psum units); LSE sums
                # in lse8 (beta = half psum unit) -> M = sum(exact) + 2*sum(ln)
                # The ACT Ln spline is only accurate near [1,2), so split
                # ln(acc) = ln(mantissa) + exponent*ln2 with DVE bit ops.
                acc = lse8s[qs][:, lo:lo + 4]
                accu = acc.bitcast(u32)
                e_u = gp.tile([128, 4], u32, tag="e_u", name="e_u", bufs=3)
                nc.vector.tensor_scalar(e_u[:], accu, 23, None, op0=SHR)
                e_f = gp.tile([128, 4], f32, tag="e_f", name="e_f", bufs=3)
                nc.vector.tensor_copy(e_f[:], e_u[:])
                m_u = gp.tile([128, 4], u32, tag="m_u", name="m_u", bufs=3)
                nc.vector.tensor_scalar(m_u[:], accu, 0x007FFFFF, 0x3F800000,
                                        op0=BAND, op1=BOR)
                lnm = gp.tile([128, 4], f32, tag="lnm", name="lnm", bufs=3)
                nc.scalar.activation(lnm[:], m_u.bitcast(f32)[:], Ln)
                ln4 = gp.tile([128, 4], f32, tag="ln4", name="ln4", bufs=3)
                nc.vector.tensor_scalar(ln4[:], e_f[:], 0.69314718056,
                                        -88.02969193, op0=MULT, op1=ADD)
                nc.vector.tensor_tensor(ln4[:], ln4[:], lnm[:], op=ADD)
                r2 = gp.tile([128, 1], f32, tag="r2", name="r2", bufs=3)
                nc.vector.reduce_sum(r2[:], ln4[:], axis=AX)
                r1 = gp.tile([128, 1], f32, tag="r1", name="r1", bufs=3)
                nc.vector.reduce_sum(
                    r1[:], m16s[qs][:, 2 * lo + 1:2 * lo + 8:2], axis=AX)
                nc.vector.tensor_scalar(Mdst[:, qs:qs + 1], r2[:], 2.0, r1[:],
                                        op0=MULT, op1=ADD)

            for qs in range(16):
                emit_msum(Mka, qs, 0)
            # (aa -> block 1, ba -> block 0); overlaps the x3-key score groups
            emit_gate_pair(Mka, [(va_sb, 1), (va_sb, 0)], "1")
            for grp in range(4, 8):            # x3-stream key images 8..15
                emit_kconv(2 * grp)
                emit_kconv(2 * grp + 1)
                for qs in range(16):
                    emit_scores(qs, grp)
            for qs in range(16):
                emit_msum(Mkb, qs, 4)
            # (ab -> block 2, bb -> block 3)
            emit_gate_pair(Mkb, [(vb_sb, 2), (vb_sb, 3)], "2")

    nc.compile()
    return nc


def get_nc():
    if "nc" not in _CACHE:
        _CACHE["nc"] = _build_nc()
    return _CACHE["nc"]


def prepare_in_maps(x4, x3, wq, bq, wk, bk, wv, bv, w128, b128):
    bf16 = ml_dtypes.bfloat16
    x4 = np.asarray(x4, np.float32)
    x3 = np.asarray(x3, np.float32)
    X4 = np.ascontiguousarray(x4.transpose(1, 0, 2, 3).reshape(C, BHW))
    X3 = np.ascontiguousarray(x3.transpose(1, 0, 2, 3).reshape(C, BHW))
    X4b = X4.astype(bf16)
    X3b = X3.astype(bf16)
    wq = np.asarray(wq, np.float32)
    wk = np.asarray(wk, np.float32)
    wv = np.asarray(wv, np.float32)
    w128 = np.asarray(w128, np.float32)
    wqT = np.ascontiguousarray(wq.T).astype(bf16)
    wkT = np.ascontiguousarray(wk.T).astype(bf16)
    wvT = np.ascontiguousarray((w128 @ wv).T)          # (256, 128) f32
    bq2 = 4.0 * np.asarray(bq, np.float32).reshape(C, 1)
    bk2 = 4.0 * np.asarray(bk, np.float32).reshape(C, 1)
    bvb = (w128 @ np.asarray(bv, np.float32)).reshape(128, 1).astype(np.float32)
    b128r = np.asarray(b128, np.float32).reshape(128, 1)

    in_maps = []
    for j in range(NCORES):
        sl = slice(j * HW, (j + 1) * HW)
        xq = np.concatenate([X4b[:, sl], X3b[:, sl]], axis=1)
        xv = np.concatenate([X4[:, sl], X3[:, sl]], axis=1)
        in_maps.append({
            "x4b": X4b, "x3b": X3b,
            "xq": np.ascontiguousarray(xq),
            "xv": np.ascontiguousarray(xv),
            "wqT": wqT, "wkT": wkT, "wvT": wvT,
            "bq": bq2, "bk": bk2, "bvb": bvb, "b128": b128r,
        })
    return in_maps


def kernel(**inputs):
    from concourse.bass_utils import run_bass_kernel_spmd

    nc = get_nc()
    in_maps = prepare_in_maps(**inputs)
    res = run_bass_kernel_spmd(nc, in_maps, core_ids=list(range(NCORES)))
    out = np.stack([res.results[c]["out"].reshape(512, 32, 32)
                    for c in range(NCORES)])
    return np.ascontiguousarray(out.astype(np.float32))


# revision 25
# speedup vs baseline: 1.1930x; 1.0489x over previous
"""Trainium2 Bass kernel for nn_AttentionModule_16398185136487.

Math (the reference reduces to this — its trailing softmax is over a size-1
axis, i.e. ones):
  out = concat([x34, a_x4, x43, b_x3], axis=1)            # (8, 512, 32, 32)
  block(qs, ks, v) = gate(qs, ks) * (w128@wv @ x_v + w128@bv) + b128
  gate(qs, ks)[b, hw] = softmax_hw( (1/8) sum_{kb} max_{khw}
                                    (Q_qs[b,hw] . K_ks[kb,khw]) / 16 )

Sharding: core j owns batch image j (its 1024 query pixels for both the x4
and x3 streams) — the per-image softmax is then fully core-local; no
collectives.  The K tensors (all 16 key images) are computed replicated on
every core from the full x4/x3 (a small 1x1 conv).

Engine plan per core:
  - Q/K convs in bf16, V conv in fp32 (weights fused host-side: w128@wv).
  - scores: 1024 bf16 matmuls (contiguous accumulate pairs over C=256) into
    paired PSUM tiles t0=[A.k0|B.k0], t1=[A.k1|B.k1]; ScalarE stages t0 to
    SBUF while the PE fills t1; a custom DVE op (TTMAX_REDUCE) consumes the
    two 512-wide operands in one pass, producing each image's max.
  - the score loop is key-image-pair OUTER (16 q-tiles inner) with the K conv
    for each image pair emitted just before its consumers, so the PE ramps up
    as soon as the first K image arrives from HBM instead of waiting for the
    whole 8 MB K stream.
  - per-image softmax without max-subtraction (logits are O(1)), gates
    broadcast to 128 partitions via K=1 PE matmuls, and a second custom DVE
    op (GMUL_BIAS) applies out = gate_row * V * (1/S) + b128 in one pass.
    Gates pair (aa, ba) — complete once the 8 x4-key images are consumed —
    runs overlapped with the x3-key score groups; only (ab, bb) is a tail.
"""

import numpy as np
import ml_dtypes

B = 8
C = 256
HW = 1024          # 32*32
BHW = B * HW       # 8192
NCORES = 8

_CACHE = {}


def _ref_ttmax(in0, in1, c0, c1, c2):
    b = np.maximum(in0.astype(np.float32), in1.astype(np.float32))
    return b, np.maximum(c0, b.reshape(b.shape[0], -1).max(axis=-1, keepdims=True))


def _ref_gmul_bias(in0, in1, c0, c1, c2):
    return (in0.astype(np.float32) * in1 * c1 + c0).astype(np.float32)


def _get_custom_ops():
    """Register two custom DVE microcode ops (the ISA-level
    TENSOR_TENSOR_REDUCE crashes this hardware, so we ship our own):

      TTMAX_REDUCE: out = max(in0, in1); accum_out = max(s0, max_k out)
      GMUL_BIAS:    out = in0 * in1 * s1 + s0     (s0, s1 per-partition APs)
    """
    if "ops" in _CACHE:
        return _CACHE["ops"]
    import concourse.dve_ops as dve_ops
    from concourse.dve_ops import DveOp
    from concourse.dve_spec import Spec, Src0, Src1, C0, C1, maxx, lower
    from concourse.dve_uop import DveOpSpec

    def register(name, spec):
        for op in dve_ops.OPS:
            if op.name == name:
                return op
        shas = {}
        for ver in ("v3", "v4"):
            shas[ver] = DveOpSpec(name=name, opcode=1,
                                  uops=lower(spec, ver=ver),
                                  rd1_en=True).sha(ver)
        op = DveOp(name, spec, subdim=False, uops_sha=shas)
        dve_ops.OPS.append(op)
        dve_ops.CUSTOM_DVE_SPECS[op.name] = op.spec
        dve_ops._SUB_OPCODE_FOR_NAME[op.name] = (
            dve_ops._CUSTOM_DVE_ROW_BASE + len(dve_ops.OPS) - 1)
        assert max(dve_ops._SUB_OPCODE_FOR_NAME.values()) < 0x20
        return op

    ttmax = register("TTMAX_REDUCE",
                     Spec(body=maxx(Src0, Src1), accum=maxx, accum_init=C0,
                          reference=_ref_ttmax))
    gmul = register("GMUL_BIAS",
                    Spec(body=Src0 * Src1 * C1 + C0,
                         reference=_ref_gmul_bias))
    _CACHE["ops"] = (ttmax, gmul)
    return _CACHE["ops"]


def _build_nc():
    from contextlib import ExitStack

    import concourse.bass as bass
    import concourse.mybir as mybir
    import concourse.tile as tile
    from concourse import bacc
    from concourse.masks import make_identity

    f32 = mybir.dt.float32
    bf16 = mybir.dt.bfloat16
    fp8 = mybir.dt.float8e4
    AX = mybir.AxisListType.X
    Exp = mybir.ActivationFunctionType.Exp
    Ident = mybir.ActivationFunctionType.Identity

    ttmax, gmul = _get_custom_ops()
    nc = bacc.Bacc("TRN2", target_bir_lowering=False, debug=False,
                   enable_asserts=False, num_devices=NCORES)

    # DRAM I/O (per core)
    x4b_ap = nc.dram_tensor("x4b", (C, BHW), bf16, kind="ExternalInput").ap()
    x3b_ap = nc.dram_tensor("x3b", (C, BHW), bf16, kind="ExternalInput").ap()
    xq_ap = nc.dram_tensor("xq", (C, 2 * HW), bf16, kind="ExternalInput").ap()
    xv_ap = nc.dram_tensor("xv", (C, 2 * HW), f32, kind="ExternalInput").ap()
    wqT_ap = nc.dram_tensor("wqT", (C, C), bf16, kind="ExternalInput").ap()
    wkT_ap = nc.dram_tensor("wkT", (C, C), bf16, kind="ExternalInput").ap()
    wvT_ap = nc.dram_tensor("wvT", (C, 128), f32, kind="ExternalInput").ap()
    bq_ap = nc.dram_tensor("bq", (C, 1), f32, kind="ExternalInput").ap()
    bk_ap = nc.dram_tensor("bk", (C, 1), f32, kind="ExternalInput").ap()
    bvb_ap = nc.dram_tensor("bvb", (128, 1), f32, kind="ExternalInput").ap()
    b128_ap = nc.dram_tensor("b128", (128, 1), f32, kind="ExternalInput").ap()
    out_ap = nc.dram_tensor("out", (512, HW), f32, kind="ExternalOutput").ap()

    SCALE_EFF = (1.0 / 16.0) / 8.0 / 16.0  # /sqrt(C), /8 mean, /16 fp8 pre-scale

    with tile.TileContext(nc) as tc:
        with ExitStack() as ctx:
            const = ctx.enter_context(tc.tile_pool(name="const", bufs=1))
            xs = ctx.enter_context(tc.tile_pool(name="xs", bufs=6))
            ps_pool = ctx.enter_context(
                tc.tile_pool(name="ps", bufs=3, space="PSUM"))
            gps_pool = ctx.enter_context(
                tc.tile_pool(name="gps", bufs=2, space="PSUM"))
            scr = ctx.enter_context(tc.tile_pool(name="scr", bufs=3))
            gp = ctx.enter_context(tc.tile_pool(name="gp", bufs=2))
            fin = ctx.enter_context(tc.tile_pool(name="fin", bufs=2))

            # ---- weights / constants (queue-critical first) ----
            wq_sb, bq_sb, xq_sb = [], [], []
            for ci in range(2):
                w = const.tile([128, C], bf16, tag=f"wq{ci}", name=f"wq{ci}")
                nc.sync.dma_start(w[:], wqT_ap[ci * 128:(ci + 1) * 128, :])
                wq_sb.append(w)
                t = const.tile([128, 2 * HW], bf16, tag=f"xq{ci}", name=f"xq{ci}")
                for nq in range(2):
                    nc.sync.dma_start(
                        t[:, nq * HW:(nq + 1) * HW],
                        xq_ap[ci * 128:(ci + 1) * 128, nq * HW:(nq + 1) * HW])
                xq_sb.append(t)
                b = const.tile([128, 1], f32, tag=f"bq{ci}", name=f"bq{ci}")
                nc.gpsimd.dma_start(b[:], bq_ap[ci * 128:(ci + 1) * 128, :])
                bq_sb.append(b)
            wk_sb, bk_sb = [], []
            for ci in range(2):
                w = const.tile([128, C], bf16, tag=f"wk{ci}", name=f"wk{ci}")
                nc.scalar.dma_start(w[:], wkT_ap[ci * 128:(ci + 1) * 128, :])
                wk_sb.append(w)
                b = const.tile([128, 1], f32, tag=f"bk{ci}", name=f"bk{ci}")
                nc.gpsimd.dma_start(b[:], bk_ap[ci * 128:(ci + 1) * 128, :])
                bk_sb.append(b)
            bvb_sb = const.tile([128, 1], f32, tag="bvb", name="bvb")
            nc.gpsimd.dma_start(bvb_sb[:], bvb_ap[:, :])
            b128_sb = const.tile([128, 1], f32, tag="b128", name="b128")
            nc.gpsimd.dma_start(b128_sb[:], b128_ap[:, :])

            ones_row = const.tile([1, 128], f32, tag="ones_row", name="ones_row")
            nc.vector.memset(ones_row[:], 1.0)
            ones_col = const.tile([128, 1], f32, tag="ones_col", name="ones_col")
            nc.vector.memset(ones_col[:], 1.0)
            ident = const.tile([128, 128], f32, tag="ident", name="ident")
            make_identity(nc, ident[:])
            neg96 = const.tile([128, 1], f32, tag="neg96", name="neg96")
            nc.vector.memset(neg96[:], -70.0)

            # residents produced on device (fp8 for the DoubleRow GEMM;
            # 2D tiles viewed as [128, cin_chunk, pix])
            q8 = const.tile([128, 2 * 2 * HW], fp8, tag="q8", name="q8")
            ka8 = const.tile([128, 2 * BHW], fp8, tag="ka8", name="ka8")
            kb8 = const.tile([128, 2 * BHW], fp8, tag="kb8", name="kb8")
            q3 = q8.rearrange("p (s n) -> p s n", s=2)
            ka3 = ka8.rearrange("p (s n) -> p s n", s=2)
            kb3 = kb8.rearrange("p (s n) -> p s n", s=2)
            va_sb = const.tile([128, HW], f32, tag="va", name="va")
            vb_sb = const.tile([128, HW], f32, tag="vb", name="vb")

            # max accumulators: m_all col = qs*16+img (odd img = exact DVE
            # max), lse_all col = qs*8+grp (ScalarE LSE exp-sums, even img)
            m_all = const.tile([128, 256], f32, tag="m_all", name="m_all")
            lse_all = const.tile([128, 128], f32, tag="lse_all", name="lse_all")
            # M pair tiles: cols 0:8 = first gate (qs 0..7), 8:16 = second
            Mka = const.tile([128, 16], f32, tag="Mka", name="Mka")  # (aa, ba)
            Mkb = const.tile([128, 16], f32, tag="Mkb", name="Mkb")  # (ab, bb)

            # ---- Q conv (bf16) ----
            for co in range(2):
                for n2 in range(2):
                    qps = ps_pool.tile([128, 1024], f32, tag="ps", name="qps")
                    for half in range(2):
                        n = n2 * 2 + half
                        for ci in range(2):
                            nc.tensor.matmul(
                                qps[:, half * 512:(half + 1) * 512],
                                wq_sb[ci][:, co * 128:(co + 1) * 128],
                                xq_sb[ci][:, n * 512:(n + 1) * 512],
                                start=(ci == 0), stop=(ci == 1))
                    nc.scalar.activation(
                        q8[:, co * 2048 + n2 * 1024:co * 2048 + (n2 + 1) * 1024],
                        qps[:, :], Ident, bias=bq_sb[co][:], scale=4.0)

            # ---- K conv for one key image (1024 cols of one stream) ----
            def emit_kconv(img):
                src_eng = nc.sync if img < 8 else nc.scalar
                src_ap = x4b_ap if img < 8 else x3b_ap
                k8 = ka8 if img < 8 else kb8
                n2 = img % 8
                xt = []
                for ci in range(2):
                    t = xs.tile([128, 1024], bf16, tag="xt", name="xt")
                    src_eng.dma_start(
                        t[:], src_ap[ci * 128:(ci + 1) * 128,
                                     n2 * 1024:(n2 + 1) * 1024])
                    xt.append(t)
                for co in range(2):
                    kps = ps_pool.tile([128, 1024], f32, tag="ps", name="kps")
                    for half in range(2):
                        for ci in range(2):
                            nc.tensor.matmul(
                                kps[:, half * 512:(half + 1) * 512],
                                wk_sb[ci][:, co * 128:(co + 1) * 128],
                                xt[ci][:, half * 512:(half + 1) * 512],
                                start=(ci == 0), stop=(ci == 1))
                    kdst = k8[:, co * BHW + n2 * 1024:
                              co * BHW + (n2 + 1) * 1024]
                    if co == 0:
                        nc.scalar.activation(kdst, kps[:, :], Ident,
                                             bias=bk_sb[co][:], scale=4.0)
                    else:
                        nc.vector.tensor_scalar(
                            kdst, kps[:, :], 4.0, bk_sb[co][:],
                            op0=mybir.AluOpType.mult,
                            op1=mybir.AluOpType.add)

            # ---- V conv (fp32, fused weights) ----
            def emit_vconv():
                wv_sb, xv_sb = [], []
                for ci in range(2):
                    w = const.tile([128, 128], f32, tag=f"wv{ci}", name=f"wv{ci}")
                    nc.gpsimd.dma_start(w[:], wvT_ap[ci * 128:(ci + 1) * 128, :])
                    wv_sb.append(w)
                    t = const.tile([128, 2 * HW], f32, tag=f"xv{ci}",
                                   name=f"xv{ci}")
                    nc.gpsimd.dma_start(t[:], xv_ap[ci * 128:(ci + 1) * 128, :])
                    xv_sb.append(t)
                for st, v_sb in ((0, va_sb), (1, vb_sb)):
                    vps = ps_pool.tile([128, 1024], f32, tag="ps", name="vps")
                    for half in range(2):
                        for ci in range(2):
                            nc.tensor.matmul(
                                vps[:, half * 512:(half + 1) * 512],
                                wv_sb[ci][:, :],
                                xv_sb[ci][:, st * HW + half * 512:
                                           st * HW + (half + 1) * 512],
                                start=(ci == 0), stop=(ci == 1))
                    nc.scalar.activation(v_sb[:, :], vps[:, :], Ident,
                                         bias=bvb_sb[:])

            # ---- scores for one (q-tile, key-image-pair) ----
            # t0 holds ALL 1024 scores of image ia, t1 of image ib.  The two
            # tiles then go to two INDEPENDENT consumers (no cross-dep, no
            # staging copy):
            #   t0 -> ScalarE log-sum-exp max (exp accum straight from PSUM;
            #         the log happens once per q-tile later; beta_eff = 16 in
            #         true-score units, so the LSE-max error is ~0.03/16 and
            #         constant shifts cancel in the softmax)
            #   t1 -> VectorE exact reduce_max from PSUM
            def emit_scores(qs, grp):
                ia, ib = grp * 2, grp * 2 + 1
                qcol = qs * 128
                t0 = ps_pool.tile([128, 1024], f32, tag="ps", name="t0")
                t1 = ps_pool.tile([128, 1024], f32, tag="ps", name="t1")

                def score_mms(tdst, img, half):
                    k3 = ka3 if img < 8 else kb3
                    kcol = (img % 8) * HW + half * 512
                    nc.tensor.matmul(
                        tdst[:, half * 512:(half + 1) * 512],
                        q3[:, :, qcol:qcol + 128],
                        k3[:, :, kcol:kcol + 512],
                        start=True, stop=True,
                        perf_mode=mybir.MatmulPerfMode.DoubleRow)

                score_mms(t0, ia, 0)
                score_mms(t0, ia, 1)
                ex = scr.tile([128, 1024], f32, tag="ex", name="ex", bufs=3)
                lcol = qs * 8 + grp
                nc.scalar.activation(ex[:], t0[:, :], Exp, bias=neg96[:],
                                     scale=0.5,
                                     accum_out=lse_all[:, lcol:lcol + 1])
                score_mms(t1, ib, 0)
                score_mms(t1, ib, 1)
                mcol = qs * 16 + ib
                nc.vector.reduce_max(m_all[:, mcol:mcol + 1], t1[:, :], axis=AX)

            # ---- batched softmax + apply for a pair of gates ----
            def emit_gate_pair(Mpair, specs, tagp):
                E2 = gp.tile([128, 16], f32, tag=f"E2{tagp}", name="E2")
                nc.scalar.activation(E2[:], Mpair[:], Exp, bias=0.0,
                                     scale=SCALE_EFF)
                sr = gp.tile([128, 2], f32, tag=f"sr{tagp}", name="sr")
                nc.vector.reduce_sum(
                    sr[:], E2.rearrange("p (g k) -> p g k", g=2), axis=AX)
                sum_ps = gps_pool.tile([128, 512], f32, tag="gps", name="sum_ps")
                nc.tensor.matmul(sum_ps[0:2, 0:1], sr[:], ones_col[:],
                                 start=True, stop=True)
                rec2 = gp.tile([2, 1], f32, tag=f"rec{tagp}", name="rec2")
                nc.vector.reciprocal(rec2[:], sum_ps[0:2, 0:1])
                tp = gps_pool.tile([128, 512], f32, tag="gps", name="tp")
                nc.tensor.transpose(tp[0:1, 0:2], rec2[:], ident[0:2, 0:2])
                recT = gp.tile([1, 2], f32, tag=f"recT{tagp}", name="recT")
                nc.scalar.copy(recT[:], tp[0:1, 0:2])
                bc = gps_pool.tile([128, 512], f32, tag="gps", name="bc")
                nc.tensor.matmul(bc[:, 0:2], ones_row[:], recT[:],
                                 start=True, stop=True)
                rsb2 = gp.tile([128, 2], f32, tag=f"rsb{tagp}", name="rsb2")
                nc.scalar.copy(rsb2[:], bc[:, 0:2])
                # transpose E (128,16) -> (16,128), flatten to a (1,2048) row
                tpe = gps_pool.tile([128, 512], f32, tag="gps", name="tpe")
                nc.tensor.transpose(tpe[0:16, 0:128], E2[:], ident[:])
                et = gp.tile([16, 128], f32, tag=f"et{tagp}", name="et")
                nc.scalar.copy(et[:], tpe[0:16, 0:128])
                grow = gp.tile([1, 2048], f32, tag=f"grow{tagp}", name="grow")
                nc.sync.dma_start(grow.rearrange("a (t p) -> a t p", t=16),
                                  et[:])
                for gidx, (v_sb, blk) in enumerate(specs):
                    out_t = fin.tile([128, HW], f32, tag="out_t", name="out_t")
                    for half in range(2):
                        gb = gps_pool.tile([128, 512], f32, tag="gps",
                                           name="gb")
                        nc.tensor.matmul(
                            gb[:, :], ones_row[:],
                            grow[0:1, gidx * 1024 + half * 512:
                                 gidx * 1024 + (half + 1) * 512],
                            start=True, stop=True)
                        nc.vector._custom_dve(
                            gmul, out=out_t[:, half * 512:(half + 1) * 512],
                            in0=gb[:, :],
                            in1=v_sb[:, half * 512:(half + 1) * 512],
                            s0=b128_sb[:], s1=rsb2[:, gidx:gidx + 1])
                    eng = nc.sync if gidx == 0 else nc.scalar
                    eng.dma_start(out_ap[blk * 128:(blk + 1) * 128, :],
                                  out_t[:])

            # ---- main schedule ----
            for grp in range(4):               # x4-stream key images 0..7
                emit_kconv(2 * grp)
                emit_kconv(2 * grp + 1)
                if grp == 0:
                    emit_vconv()
                for qs in range(16):
                    emit_scores(qs, grp)
            Ln = mybir.ActivationFunctionType.Ln
            MULT, ADD = mybir.AluOpType.mult, mybir.AluOpType.add

            u32 = mybir.dt.uint32
            SHR = mybir.AluOpType.logical_shift_right
            BAND = mybir.AluOpType.bitwise_and
            BOR = mybir.AluOpType.bitwise_or

            def emit_msum(Mdst, lo):
                # One batched pass for all 16 q-tiles.  Exact maxes live in
                # odd m_all columns (psum units); LSE exp-sums in lse_all
                # (beta = half psum unit): M = sum(exact) + 2*sum(ln(acc)).
                # The ACT Ln spline is only accurate near [1,2), so split
                # ln(acc) = ln(mantissa) + exponent*ln2 with DVE bit ops.
                accv = lse_all.bitcast(u32).rearrange(
                    "p (q g) -> p q g", q=16)[:, :, lo:lo + 4]
                e_u = gp.tile([128, 64], u32, tag="e_u", name="e_u")
                e_uv = e_u.rearrange("p (q g) -> p q g", q=16)
                nc.vector.tensor_scalar(e_uv, accv, 23, None, op0=SHR)
                e_f = gp.tile([128, 64], f32, tag="e_f", name="e_f")
                nc.vector.tensor_copy(e_f[:], e_u[:])
                m_u = gp.tile([128, 64], u32, tag="m_u", name="m_u")
                m_uv = m_u.rearrange("p (q g) -> p q g", q=16)
                nc.vector.tensor_scalar(m_uv, accv, 0x007FFFFF, 0x3F800000,
                                        op0=BAND, op1=BOR)
                lnm = gp.tile([128, 64], f32, tag="lnm", name="lnm")
                nc.scalar.activation(lnm[:], m_u.bitcast(f32)[:], Ln)
                ln4 = gp.tile([128, 64], f32, tag="ln4", name="ln4")
                nc.vector.tensor_scalar(ln4[:], e_f[:], 0.69314718056,
                                        -88.02969193, op0=MULT, op1=ADD)
                nc.vector.tensor_tensor(ln4[:], ln4[:], lnm[:], op=ADD)
                r2 = gp.tile([128, 16], f32, tag="r2", name="r2")
                nc.vector.reduce_sum(
                    r2[:], ln4.rearrange("p (q g) -> p q g", q=16), axis=AX)
                r1 = gp.tile([128, 16], f32, tag="r1", name="r1")
                nc.vector.reduce_sum(
                    r1[:], m_all.rearrange("p (q i) -> p q i", q=16)
                    [:, :, 2 * lo + 1:2 * lo + 8:2], axis=AX)
                nc.vector.tensor_scalar(r2[:], r2[:], 2.0, None, op0=MULT)
                nc.vector.tensor_tensor(Mdst[:], r2[:], r1[:], op=ADD)

            emit_msum(Mka, 0)
            # (aa -> block 1, ba -> block 0); overlaps the x3-key score groups
            emit_gate_pair(Mka, [(va_sb, 1), (va_sb, 0)], "1")
            for grp in range(4, 8):            # x3-stream key images 8..15
                emit_kconv(2 * grp)
                emit_kconv(2 * grp + 1)
                for qs in range(16):
                    emit_scores(qs, grp)
            emit_msum(Mkb, 4)
            # (ab -> block 2, bb -> block 3)
            emit_gate_pair(Mkb, [(vb_sb, 2), (vb_sb, 3)], "2")

    nc.compile()
    return nc


def get_nc():
    if "nc" not in _CACHE:
        _CACHE["nc"] = _build_nc()
    return _CACHE["nc"]


def prepare_in_maps(x4, x3, wq, bq, wk, bk, wv, bv, w128, b128):
    bf16 = ml_dtypes.bfloat16
    x4 = np.asarray(x4, np.float32)
    x3 = np.asarray(x3, np.float32)
    X4 = np.ascontiguousarray(x4.transpose(1, 0, 2, 3).reshape(C, BHW))
    X3 = np.ascontiguousarray(x3.transpose(1, 0, 2, 3).reshape(C, BHW))
    X4b = X4.astype(bf16)
    X3b = X3.astype(bf16)
    wq = np.asarray(wq, np.float32)
    wk = np.asarray(wk, np.float32)
    wv = np.asarray(wv, np.float32)
    w128 = np.asarray(w128, np.float32)
    wqT = np.ascontiguousarray(wq.T).astype(bf16)
    wkT = np.ascontiguousarray(wk.T).astype(bf16)
    wvT = np.ascontiguousarray((w128 @ wv).T)          # (256, 128) f32
    bq2 = 4.0 * np.asarray(bq, np.float32).reshape(C, 1)
    bk2 = 4.0 * np.asarray(bk, np.float32).reshape(C, 1)
    bvb = (w128 @ np.asarray(bv, np.float32)).reshape(128, 1).astype(np.float32)
    b128r = np.asarray(b128, np.float32).reshape(128, 1)

    in_maps = []
    for j in range(NCORES):
        sl = slice(j * HW, (j + 1) * HW)
        xq = np.concatenate([X4b[:, sl], X3b[:, sl]], axis=1)
        xv = np.concatenate([X4[:, sl], X3[:, sl]], axis=1)
        in_maps.append({
            "x4b": X4b, "x3b": X3b,
            "xq": np.ascontiguousarray(xq),
            "xv": np.ascontiguousarray(xv),
            "wqT": wqT, "wkT": wkT, "wvT": wvT,
            "bq": bq2, "bk": bk2, "bvb": bvb, "b128": b128r,
        })
    return in_maps


def kernel(**inputs):
    from concourse.bass_utils import run_bass_kernel_spmd

    nc = get_nc()
    in_maps = prepare_in_maps(**inputs)
    res = run_bass_kernel_spmd(nc, in_maps, core_ids=list(range(NCORES)))
    out = np.stack([res.results[c]["out"].reshape(512, 32, 32)
                    for c in range(NCORES)])
    return np.ascontiguousarray(out.astype(np.float32))


# revision 26
# speedup vs baseline: 1.2322x; 1.0328x over previous
"""Trainium2 Bass kernel for nn_AttentionModule_16398185136487.

Math (the reference reduces to this — its trailing softmax is over a size-1
axis, i.e. ones):
  out = concat([x34, a_x4, x43, b_x3], axis=1)            # (8, 512, 32, 32)
  block(qs, ks, v) = gate(qs, ks) * (w128@wv @ x_v + w128@bv) + b128
  gate(qs, ks)[b, hw] = softmax_hw( (1/8) sum_{kb} max_{khw}
                                    (Q_qs[b,hw] . K_ks[kb,khw]) / 16 )

Sharding: core j owns batch image j (its 1024 query pixels for both the x4
and x3 streams) — the per-image softmax is then fully core-local; no
collectives.  The K tensors (all 16 key images) are computed replicated on
every core from the full x4/x3 (a small 1x1 conv).

Engine plan per core:
  - Q/K convs in bf16, V conv in fp32 (weights fused host-side: w128@wv).
  - scores: 1024 bf16 matmuls (contiguous accumulate pairs over C=256) into
    paired PSUM tiles t0=[A.k0|B.k0], t1=[A.k1|B.k1]; ScalarE stages t0 to
    SBUF while the PE fills t1; a custom DVE op (TTMAX_REDUCE) consumes the
    two 512-wide operands in one pass, producing each image's max.
  - the score loop is key-image-pair OUTER (16 q-tiles inner) with the K conv
    for each image pair emitted just before its consumers, so the PE ramps up
    as soon as the first K image arrives from HBM instead of waiting for the
    whole 8 MB K stream.
  - per-image softmax without max-subtraction (logits are O(1)), gates
    broadcast to 128 partitions via K=1 PE matmuls, and a second custom DVE
    op (GMUL_BIAS) applies out = gate_row * V * (1/S) + b128 in one pass.
    Gates pair (aa, ba) — complete once the 8 x4-key images are consumed —
    runs overlapped with the x3-key score groups; only (ab, bb) is a tail.
"""

import numpy as np
import ml_dtypes

B = 8
C = 256
HW = 1024          # 32*32
BHW = B * HW       # 8192
NCORES = 8

_CACHE = {}


def _ref_ttmax(in0, in1, c0, c1, c2):
    b = np.maximum(in0.astype(np.float32), in1.astype(np.float32))
    return b, np.maximum(c0, b.reshape(b.shape[0], -1).max(axis=-1, keepdims=True))


def _ref_gmul_bias(in0, in1, c0, c1, c2):
    return (in0.astype(np.float32) * in1 * c1 + c0).astype(np.float32)


def _get_custom_ops():
    """Register two custom DVE microcode ops (the ISA-level
    TENSOR_TENSOR_REDUCE crashes this hardware, so we ship our own):

      TTMAX_REDUCE: out = max(in0, in1); accum_out = max(s0, max_k out)
      GMUL_BIAS:    out = in0 * in1 * s1 + s0     (s0, s1 per-partition APs)
    """
    if "ops" in _CACHE:
        return _CACHE["ops"]
    import concourse.dve_ops as dve_ops
    from concourse.dve_ops import DveOp
    from concourse.dve_spec import Spec, Src0, Src1, C0, C1, maxx, lower
    from concourse.dve_uop import DveOpSpec

    def register(name, spec):
        for op in dve_ops.OPS:
            if op.name == name:
                return op
        shas = {}
        for ver in ("v3", "v4"):
            shas[ver] = DveOpSpec(name=name, opcode=1,
                                  uops=lower(spec, ver=ver),
                                  rd1_en=True).sha(ver)
        op = DveOp(name, spec, subdim=False, uops_sha=shas)
        dve_ops.OPS.append(op)
        dve_ops.CUSTOM_DVE_SPECS[op.name] = op.spec
        dve_ops._SUB_OPCODE_FOR_NAME[op.name] = (
            dve_ops._CUSTOM_DVE_ROW_BASE + len(dve_ops.OPS) - 1)
        assert max(dve_ops._SUB_OPCODE_FOR_NAME.values()) < 0x20
        return op

    ttmax = register("TTMAX_REDUCE",
                     Spec(body=maxx(Src0, Src1), accum=maxx, accum_init=C0,
                          reference=_ref_ttmax))
    gmul = register("GMUL_BIAS",
                    Spec(body=Src0 * Src1 * C1 + C0,
                         reference=_ref_gmul_bias))
    _CACHE["ops"] = (ttmax, gmul)
    return _CACHE["ops"]


def _build_nc():
    from contextlib import ExitStack

    import concourse.bass as bass
    import concourse.mybir as mybir
    import concourse.tile as tile
    from concourse import bacc
    from concourse.masks import make_identity

    f32 = mybir.dt.float32
    bf16 = mybir.dt.bfloat16
    fp8 = mybir.dt.float8e4
    AX = mybir.AxisListType.X
    Exp = mybir.ActivationFunctionType.Exp
    Ident = mybir.ActivationFunctionType.Identity

    ttmax, gmul = _get_custom_ops()
    nc = bacc.Bacc("TRN2", target_bir_lowering=False, debug=False,
                   enable_asserts=False, num_devices=NCORES)

    # DRAM I/O (per core)
    x4b_ap = nc.dram_tensor("x4b", (C, BHW), bf16, kind="ExternalInput").ap()
    x3b_ap = nc.dram_tensor("x3b", (C, BHW), bf16, kind="ExternalInput").ap()
    xq_ap = nc.dram_tensor("xq", (C, 2 * HW), bf16, kind="ExternalInput").ap()
    xv_ap = nc.dram_tensor("xv", (C, 2 * HW), f32, kind="ExternalInput").ap()
    wqT_ap = nc.dram_tensor("wqT", (C, C), bf16, kind="ExternalInput").ap()
    wkT_ap = nc.dram_tensor("wkT", (C, C), bf16, kind="ExternalInput").ap()
    wvT_ap = nc.dram_tensor("wvT", (C, 128), f32, kind="ExternalInput").ap()
    bq_ap = nc.dram_tensor("bq", (C, 1), f32, kind="ExternalInput").ap()
    bk_ap = nc.dram_tensor("bk", (C, 1), f32, kind="ExternalInput").ap()
    bvb_ap = nc.dram_tensor("bvb", (128, 1), f32, kind="ExternalInput").ap()
    b128_ap = nc.dram_tensor("b128", (128, 1), f32, kind="ExternalInput").ap()
    out_ap = nc.dram_tensor("out", (512, HW), f32, kind="ExternalOutput").ap()

    SCALE_EFF = (1.0 / 16.0) / 8.0 / 16.0  # /sqrt(C), /8 mean, /16 fp8 pre-scale

    with tile.TileContext(nc) as tc:
        with ExitStack() as ctx:
            const = ctx.enter_context(tc.tile_pool(name="const", bufs=1))
            xs = ctx.enter_context(tc.tile_pool(name="xs", bufs=6))
            ps_pool = ctx.enter_context(
                tc.tile_pool(name="ps", bufs=4, space="PSUM"))
            scr = ctx.enter_context(tc.tile_pool(name="scr", bufs=3))
            gp = ctx.enter_context(tc.tile_pool(name="gp", bufs=2))
            fin = ctx.enter_context(tc.tile_pool(name="fin", bufs=2))

            # ---- weights / constants (queue-critical first) ----
            wq_sb, bq_sb, xq_sb = [], [], []
            for ci in range(2):
                w = const.tile([128, C], bf16, tag=f"wq{ci}", name=f"wq{ci}")
                nc.sync.dma_start(w[:], wqT_ap[ci * 128:(ci + 1) * 128, :])
                wq_sb.append(w)
                t = const.tile([128, 2 * HW], bf16, tag=f"xq{ci}",
                               name=f"xq{ci}")
                xq_sb.append(t)
                b = const.tile([128, 1], f32, tag=f"bq{ci}", name=f"bq{ci}")
                nc.gpsimd.dma_start(b[:], bq_ap[ci * 128:(ci + 1) * 128, :])
                bq_sb.append(b)
            for nq in range(2):
                for ci in range(2):
                    nc.sync.dma_start(
                        xq_sb[ci][:, nq * HW:(nq + 1) * HW],
                        xq_ap[ci * 128:(ci + 1) * 128, nq * HW:(nq + 1) * HW])
            wk_sb, bk_sb = [], []
            for ci in range(2):
                w = const.tile([128, C], bf16, tag=f"wk{ci}", name=f"wk{ci}")
                nc.scalar.dma_start(w[:], wkT_ap[ci * 128:(ci + 1) * 128, :])
                wk_sb.append(w)
                b = const.tile([128, 1], f32, tag=f"bk{ci}", name=f"bk{ci}")
                nc.gpsimd.dma_start(b[:], bk_ap[ci * 128:(ci + 1) * 128, :])
                bk_sb.append(b)
            bvb_sb = const.tile([128, 1], f32, tag="bvb", name="bvb")
            nc.gpsimd.dma_start(bvb_sb[:], bvb_ap[:, :])
            b128_sb = const.tile([128, 1], f32, tag="b128", name="b128")
            nc.gpsimd.dma_start(b128_sb[:], b128_ap[:, :])

            ones_row = const.tile([1, 128], f32, tag="ones_row", name="ones_row")
            nc.vector.memset(ones_row[:], 1.0)
            ones_col = const.tile([128, 1], f32, tag="ones_col", name="ones_col")
            nc.vector.memset(ones_col[:], 1.0)
            ident = const.tile([128, 128], f32, tag="ident", name="ident")
            make_identity(nc, ident[:])
            neg96 = const.tile([128, 1], f32, tag="neg96", name="neg96")
            nc.vector.memset(neg96[:], -70.0)

            # residents produced on device (fp8 for the DoubleRow GEMM;
            # 2D tiles viewed as [128, cin_chunk, pix])
            q8 = const.tile([128, 2 * 2 * HW], fp8, tag="q8", name="q8")
            ka8 = const.tile([128, 2 * BHW], fp8, tag="ka8", name="ka8")
            kb8 = const.tile([128, 2 * BHW], fp8, tag="kb8", name="kb8")
            q3 = q8.rearrange("p (s n) -> p s n", s=2)
            ka3 = ka8.rearrange("p (s n) -> p s n", s=2)
            kb3 = kb8.rearrange("p (s n) -> p s n", s=2)
            va_sb = const.tile([128, HW], f32, tag="va", name="va")
            vb_sb = const.tile([128, HW], f32, tag="vb", name="vb")

            # max accumulators: m_all col = qs*16+img (odd img = exact DVE
            # max), lse_all col = qs*8+grp (ScalarE LSE exp-sums, even img)
            m_all = const.tile([128, 256], f32, tag="m_all", name="m_all")
            lse_all = const.tile([128, 128], f32, tag="lse_all", name="lse_all")
            # M pair tiles: cols 0:8 = first gate (qs 0..7), 8:16 = second
            Mka = const.tile([128, 16], f32, tag="Mka", name="Mka")  # (aa, ba)
            Mkb = const.tile([128, 16], f32, tag="Mkb", name="Mkb")  # (ab, bb)

            # ---- Q conv (bf16) ----
            for co in range(2):
                for n2 in range(2):
                    qps = ps_pool.tile([128, 1024], f32, tag="ps", name="qps")
                    for half in range(2):
                        n = n2 * 2 + half
                        for ci in range(2):
                            nc.tensor.matmul(
                                qps[:, half * 512:(half + 1) * 512],
                                wq_sb[ci][:, co * 128:(co + 1) * 128],
                                xq_sb[ci][:, n * 512:(n + 1) * 512],
                                start=(ci == 0), stop=(ci == 1))
                    nc.scalar.activation(
                        q8[:, co * 2048 + n2 * 1024:co * 2048 + (n2 + 1) * 1024],
                        qps[:, :], Ident, bias=bq_sb[co][:], scale=4.0)

            # ---- K conv for one key image (1024 cols of one stream) ----
            def emit_kconv(img):
                src_eng = nc.sync if img < 8 else nc.scalar
                src_ap = x4b_ap if img < 8 else x3b_ap
                k8 = ka8 if img < 8 else kb8
                n2 = img % 8
                xt = []
                for ci in range(2):
                    t = xs.tile([128, 1024], bf16, tag="xt", name="xt")
                    src_eng.dma_start(
                        t[:], src_ap[ci * 128:(ci + 1) * 128,
                                     n2 * 1024:(n2 + 1) * 1024])
                    xt.append(t)
                for co in range(2):
                    kps = ps_pool.tile([128, 1024], f32, tag="ps", name="kps")
                    for half in range(2):
                        for ci in range(2):
                            nc.tensor.matmul(
                                kps[:, half * 512:(half + 1) * 512],
                                wk_sb[ci][:, co * 128:(co + 1) * 128],
                                xt[ci][:, half * 512:(half + 1) * 512],
                                start=(ci == 0), stop=(ci == 1))
                    kdst = k8[:, co * BHW + n2 * 1024:
                              co * BHW + (n2 + 1) * 1024]
                    if co == 0:
                        nc.scalar.activation(kdst, kps[:, :], Ident,
                                             bias=bk_sb[co][:], scale=4.0)
                    else:
                        nc.vector.tensor_scalar(
                            kdst, kps[:, :], 4.0, bk_sb[co][:],
                            op0=mybir.AluOpType.mult,
                            op1=mybir.AluOpType.add)

            # ---- V conv (fp32, fused weights) ----
            def emit_vconv():
                wv_sb, xv_sb = [], []
                for ci in range(2):
                    w = const.tile([128, 128], f32, tag=f"wv{ci}", name=f"wv{ci}")
                    nc.gpsimd.dma_start(w[:], wvT_ap[ci * 128:(ci + 1) * 128, :])
                    wv_sb.append(w)
                    t = const.tile([128, 2 * HW], f32, tag=f"xv{ci}",
                                   name=f"xv{ci}")
                    nc.gpsimd.dma_start(t[:], xv_ap[ci * 128:(ci + 1) * 128, :])
                    xv_sb.append(t)
                for st, v_sb in ((0, va_sb), (1, vb_sb)):
                    vps = ps_pool.tile([128, 1024], f32, tag="ps", name="vps")
                    for half in range(2):
                        for ci in range(2):
                            nc.tensor.matmul(
                                vps[:, half * 512:(half + 1) * 512],
                                wv_sb[ci][:, :],
                                xv_sb[ci][:, st * HW + half * 512:
                                           st * HW + (half + 1) * 512],
                                start=(ci == 0), stop=(ci == 1))
                    nc.scalar.activation(v_sb[:, :], vps[:, :], Ident,
                                         bias=bvb_sb[:])

            # ---- scores for one (q-tile, key-image-pair) ----
            # t0 holds ALL 1024 scores of image ia, t1 of image ib.  The two
            # tiles then go to two INDEPENDENT consumers (no cross-dep, no
            # staging copy):
            #   t0 -> ScalarE log-sum-exp max (exp accum straight from PSUM;
            #         the log happens once per q-tile later; beta_eff = 16 in
            #         true-score units, so the LSE-max error is ~0.03/16 and
            #         constant shifts cancel in the softmax)
            #   t1 -> VectorE exact reduce_max from PSUM
            def emit_scores(qs, grp):
                ia, ib = grp * 2, grp * 2 + 1
                qcol = qs * 128
                t0 = ps_pool.tile([128, 1024], f32, tag="ps", name="t0")
                t1 = ps_pool.tile([128, 1024], f32, tag="ps", name="t1")

                def score_mms(tdst, img, half):
                    k3 = ka3 if img < 8 else kb3
                    kcol = (img % 8) * HW + half * 512
                    nc.tensor.matmul(
                        tdst[:, half * 512:(half + 1) * 512],
                        q3[:, :, qcol:qcol + 128],
                        k3[:, :, kcol:kcol + 512],
                        start=True, stop=True,
                        perf_mode=mybir.MatmulPerfMode.DoubleRow)

                score_mms(t0, ia, 0)
                score_mms(t0, ia, 1)
                ex = scr.tile([128, 1024], f32, tag="ex", name="ex", bufs=3)
                lcol = qs * 8 + grp
                nc.scalar.activation(ex[:], t0[:, :], Exp, bias=neg96[:],
                                     scale=0.5,
                                     accum_out=lse_all[:, lcol:lcol + 1])
                score_mms(t1, ib, 0)
                score_mms(t1, ib, 1)
                mcol = qs * 16 + ib
                nc.vector.reduce_max(m_all[:, mcol:mcol + 1], t1[:, :], axis=AX)

            # ---- batched softmax + apply for a pair of gates ----
            def emit_gate_pair(Mpair, specs, tagp):
                E2 = gp.tile([128, 16], f32, tag=f"E2{tagp}", name="E2")
                nc.scalar.activation(E2[:], Mpair[:], Exp, bias=0.0,
                                     scale=SCALE_EFF)
                sr = gp.tile([128, 2], f32, tag=f"sr{tagp}", name="sr")
                nc.vector.reduce_sum(
                    sr[:], E2.rearrange("p (g k) -> p g k", g=2), axis=AX)
                sum_ps = ps_pool.tile([128, 512], f32, tag="ps", name="sum_ps")
                nc.tensor.matmul(sum_ps[0:2, 0:1], sr[:], ones_col[:],
                                 start=True, stop=True)
                rec2 = gp.tile([2, 1], f32, tag=f"rec{tagp}", name="rec2")
                nc.vector.reciprocal(rec2[:], sum_ps[0:2, 0:1])
                tp = ps_pool.tile([128, 512], f32, tag="ps", name="tp")
                nc.tensor.transpose(tp[0:1, 0:2], rec2[:], ident[0:2, 0:2])
                recT = gp.tile([1, 2], f32, tag=f"recT{tagp}", name="recT")
                nc.scalar.copy(recT[:], tp[0:1, 0:2])
                bc = ps_pool.tile([128, 512], f32, tag="ps", name="bc")
                nc.tensor.matmul(bc[:, 0:2], ones_row[:], recT[:],
                                 start=True, stop=True)
                rsb2 = gp.tile([128, 2], f32, tag=f"rsb{tagp}", name="rsb2")
                nc.scalar.copy(rsb2[:], bc[:, 0:2])
                # transpose E (128,16) -> (16,128), flatten to a (1,2048) row
                tpe = ps_pool.tile([128, 512], f32, tag="ps", name="tpe")
                nc.tensor.transpose(tpe[0:16, 0:128], E2[:], ident[:])
                et = gp.tile([16, 128], f32, tag=f"et{tagp}", name="et")
                nc.scalar.copy(et[:], tpe[0:16, 0:128])
                grow = gp.tile([1, 2048], f32, tag=f"grow{tagp}", name="grow")
                nc.sync.dma_start(grow.rearrange("a (t p) -> a t p", t=16),
                                  et[:])
                for gidx, (v_sb, blk) in enumerate(specs):
                    out_t = fin.tile([128, HW], f32, tag="out_t", name="out_t")
                    for half in range(2):
                        gb = ps_pool.tile([128, 512], f32, tag="ps",
                                           name="gb")
                        nc.tensor.matmul(
                            gb[:, :], ones_row[:],
                            grow[0:1, gidx * 1024 + half * 512:
                                 gidx * 1024 + (half + 1) * 512],
                            start=True, stop=True)
                        nc.vector._custom_dve(
                            gmul, out=out_t[:, half * 512:(half + 1) * 512],
                            in0=gb[:, :],
                            in1=v_sb[:, half * 512:(half + 1) * 512],
                            s0=b128_sb[:], s1=rsb2[:, gidx:gidx + 1])
                    eng = nc.sync if gidx == 0 else nc.scalar
                    eng.dma_start(out_ap[blk * 128:(blk + 1) * 128, :],
                                  out_t[:])

            # ---- main schedule ----
            for grp in range(4):               # x4-stream key images 0..7
                emit_kconv(2 * grp)
                emit_kconv(2 * grp + 1)
                if grp == 0:
                    emit_vconv()
                for qs in range(16):
                    emit_scores(qs, grp)
            Ln = mybir.ActivationFunctionType.Ln
            MULT, ADD = mybir.AluOpType.mult, mybir.AluOpType.add

            u32 = mybir.dt.uint32
            SHR = mybir.AluOpType.logical_shift_right
            BAND = mybir.AluOpType.bitwise_and
            BOR = mybir.AluOpType.bitwise_or

            def emit_msum(Mdst, lo):
                # One batched pass for all 16 q-tiles.  Exact maxes live in
                # odd m_all columns (psum units); LSE exp-sums in lse_all
                # (beta = half psum unit): M = sum(exact) + 2*sum(ln(acc)).
                # The ACT Ln spline is only accurate near [1,2), so split
                # ln(acc) = ln(mantissa) + exponent*ln2 with DVE bit ops.
                accv = lse_all.bitcast(u32).rearrange(
                    "p (q g) -> p q g", q=16)[:, :, lo:lo + 4]
                e_u = gp.tile([128, 64], u32, tag="e_u", name="e_u")
                e_uv = e_u.rearrange("p (q g) -> p q g", q=16)
                nc.vector.tensor_scalar(e_uv, accv, 23, None, op0=SHR)
                e_f = gp.tile([128, 64], f32, tag="e_f", name="e_f")
                nc.vector.tensor_copy(e_f[:], e_u[:])
                m_u = gp.tile([128, 64], u32, tag="m_u", name="m_u")
                m_uv = m_u.rearrange("p (q g) -> p q g", q=16)
                nc.vector.tensor_scalar(m_uv, accv, 0x007FFFFF, 0x3F800000,
                                        op0=BAND, op1=BOR)
                lnm = gp.tile([128, 64], f32, tag="lnm", name="lnm")
                nc.scalar.activation(lnm[:], m_u.bitcast(f32)[:], Ln)
                ln4 = gp.tile([128, 64], f32, tag="ln4", name="ln4")
                nc.vector.tensor_scalar(ln4[:], e_f[:], 0.69314718056,
                                        -88.02969193, op0=MULT, op1=ADD)
                nc.vector.tensor_tensor(ln4[:], ln4[:], lnm[:], op=ADD)
                r2 = gp.tile([128, 16], f32, tag="r2", name="r2")
                nc.vector.reduce_sum(
                    r2[:], ln4.rearrange("p (q g) -> p q g", q=16), axis=AX)
                r1 = gp.tile([128, 16], f32, tag="r1", name="r1")
                nc.vector.reduce_sum(
                    r1[:], m_all.rearrange("p (q i) -> p q i", q=16)
                    [:, :, 2 * lo + 1:2 * lo + 8:2], axis=AX)
                nc.vector.tensor_scalar(r2[:], r2[:], 2.0, None, op0=MULT)
                nc.vector.tensor_tensor(Mdst[:], r2[:], r1[:], op=ADD)

            emit_msum(Mka, 0)
            # (aa -> block 1, ba -> block 0); overlaps the x3-key score groups
            emit_gate_pair(Mka, [(va_sb, 1), (va_sb, 0)], "1")
            for grp in range(4, 8):            # x3-stream key images 8..15
                emit_kconv(2 * grp)
                emit_kconv(2 * grp + 1)
                for qs in range(16):
                    emit_scores(qs, grp)
            emit_msum(Mkb, 4)
            # (ab -> block 2, bb -> block 3)
            emit_gate_pair(Mkb, [(vb_sb, 2), (vb_sb, 3)], "2")

    nc.compile()
    return nc


def get_nc():
    if "nc" not in _CACHE:
        _CACHE["nc"] = _build_nc()
    return _CACHE["nc"]


def prepare_in_maps(x4, x3, wq, bq, wk, bk, wv, bv, w128, b128):
    bf16 = ml_dtypes.bfloat16
    x4 = np.asarray(x4, np.float32)
    x3 = np.asarray(x3, np.float32)
    X4 = np.ascontiguousarray(x4.transpose(1, 0, 2, 3).reshape(C, BHW))
    X3 = np.ascontiguousarray(x3.transpose(1, 0, 2, 3).reshape(C, BHW))
    X4b = X4.astype(bf16)
    X3b = X3.astype(bf16)
    wq = np.asarray(wq, np.float32)
    wk = np.asarray(wk, np.float32)
    wv = np.asarray(wv, np.float32)
    w128 = np.asarray(w128, np.float32)
    wqT = np.ascontiguousarray(wq.T).astype(bf16)
    wkT = np.ascontiguousarray(wk.T).astype(bf16)
    wvT = np.ascontiguousarray((w128 @ wv).T)          # (256, 128) f32
    bq2 = 4.0 * np.asarray(bq, np.float32).reshape(C, 1)
    bk2 = 4.0 * np.asarray(bk, np.float32).reshape(C, 1)
    bvb = (w128 @ np.asarray(bv, np.float32)).reshape(128, 1).astype(np.float32)
    b128r = np.asarray(b128, np.float32).reshape(128, 1)

    in_maps = []
    for j in range(NCORES):
        sl = slice(j * HW, (j + 1) * HW)
        xq = np.concatenate([X4b[:, sl], X3b[:, sl]], axis=1)
        xv = np.concatenate([X4[:, sl], X3[:, sl]], axis=1)
        in_maps.append({
            "x4b": X4b, "x3b": X3b,
            "xq": np.ascontiguousarray(xq),
            "xv": np.ascontiguousarray(xv),
            "wqT": wqT, "wkT": wkT, "wvT": wvT,
            "bq": bq2, "bk": bk2, "bvb": bvb, "b128": b128r,
        })
    return in_maps


def kernel(**inputs):
    from concourse.bass_utils import run_bass_kernel_spmd

    nc = get_nc()
    in_maps = prepare_in_maps(**inputs)
    res = run_bass_kernel_spmd(nc, in_maps, core_ids=list(range(NCORES)))
    out = np.stack([res.results[c]["out"].reshape(512, 32, 32)
                    for c in range(NCORES)])
    return np.ascontiguousarray(out.astype(np.float32))
